# revision 16
# baseline (speedup 1.0000x reference)
"""BlockAttentionResidual Trainium2 kernel (v2: fp16, 32-token groups).

Math per token t over NB=9 blocks (8 full + 1 partial), D=1024:
    rq[n,t]   = (ssq[n,t]/D + eps)^(-1/2)
    logit     = (sum_d w2[d]*v[n,t,d]) * rq,   w2 = proj_w*norm_w
    w[n,t]    = softmax_n(logit)
    h[t,d]    = sum_n w[n,t]*v[n,t,d]

Sharding: B*T = 8192 tokens -> 1024 tokens/core on 8 cores.
Per core: 8 superquads (SQ) of 128 tokens = 4 groups x 32 tokens.
fp16 data path (host-side cast; harness tolerance is 2e-2, fp16 keeps
rel err ~1e-3), fp32 accumulation in PSUM / stats.

Layout per SQ (partition dim first):
  slabA[p = n*32+t', c = g*1024+d]  n in 0..3   [128, 4096]
  slabB: same for n in 4..7                      [128, 4096]
  slabP[p = g*32+t', d]  partial block           [128, 1024]

Stats: ssq via ACT Square+accum_out, dot via DVE stt+accum_out, into
stats[:, 0:9]=ssq / [:, 9:18]=dot (order: A g0-3, B g0-3, P).
Softmax over n via PE one-hot matmuls (Z accumulated in PSUM):
  Z[t',g] = ohA^T@eA + ohA^T@eB + (ohA*e8)^T@gsel ; rz = 1/Z
h accumulated UNNORMALIZED in PSUM (weights = raw e values):
  per (group, 512-col half): 3 accumulated matmuls
  (lhsT = ohA8*e masks for A/B; diag(e8) built from I128 for P).
Normalization by 1/Z happens in the PSUM->SBUF copy:
  rzcol[p = g*32+t'] = rz[t',g] (via rzsel matmul), then
  h_sb = hpage * rzcol (ACT Copy with scale AP / DVE tensor_scalar).
PSUM pages are fully packed (128 tokens/page) so the copy and the
output DMA are contiguous [128, 1024].
Three-stage software pipeline: stats(i) | softmax+mm(i-1) | copy+out(i-2).
"""

import os
import sys
import numpy as np

for _p in ("/opt/trn_rl_repo", "/root/.axon_site/_ro/trn_rl_repo"):
    if os.path.isdir(_p) and _p not in sys.path:
        sys.path.append(_p)

N_CORES = 8
N, B, T, D = 8, 4, 2048, 1024
EPS = 1e-6
TOK = (B * T) // N_CORES          # 1024 tokens per core
TPG = 32                          # tokens per group
NG = 4                            # groups per superquad
SQTOK = TPG * NG                  # 128 tokens per superquad
NSQ = TOK // SQTOK                # 8 superquads per core

# knobs for ACT/DVE balance
COPY_DVE = int(os.environ.get("BLOCKATTN_COPY_DVE", "448"))
SSQ_DVE = int(os.environ.get("BLOCKATTN_SSQ_DVE", "0"))  # of 9 ssq units -> DVE
DTYPE = os.environ.get("BLOCKATTN_DTYPE", "bf16")  # bf16 | fp16
ACT_SET = "natural_log_exp_and_others"

_CACHE = {}


def _patch_act_tables():
    """Make every activation func this kernel uses resolve to one table set
    (ACT_SET), so bacc emits a single ACT_TABLE_LOAD instead of thrashing
    between sets on every Ln/Exp/Square transition."""
    import concourse.bacc as bacc_mod
    import concourse.hw_specs as hw_specs
    from concourse import mybir

    if getattr(bacc_mod, "_blockattn_act_patch", False):
        return
    AF = mybir.ActivationFunctionType
    mine = {AF.Square, AF.Exp, AF.Ln, AF.Copy, AF.Identity}
    orig = hw_specs.get_activation_tables

    def patched(arch):
        t = dict(orig(arch))
        assert ACT_SET in t and mine <= t[ACT_SET], (ACT_SET, t.get(ACT_SET))
        return {
            name: (funcs if name == ACT_SET else funcs - mine)
            for name, funcs in t.items()
        }

    bacc_mod.get_activation_tables = patched
    bacc_mod._blockattn_act_patch = True


def build_nc():
    import concourse.bacc as bacc
    import concourse.tile as tile
    from concourse import mybir

    _patch_act_tables()

    f32 = mybir.dt.float32
    f16 = mybir.dt.bfloat16 if DTYPE == "bf16" else mybir.dt.float16
    AF = mybir.ActivationFunctionType
    OP = mybir.AluOpType

    nc = bacc.Bacc("TRN2", target_bir_lowering=False, debug=False)

    CW = D + TPG + 8 * TPG + NG + 128  # packed 128-row consts
    slabA_d = nc.dram_tensor("slabA", [NSQ, 128, NG * D], f16, kind="ExternalInput")
    slabB_d = nc.dram_tensor("slabB", [NSQ, 128, NG * D], f16, kind="ExternalInput")
    slabP_d = nc.dram_tensor("slabP", [NSQ, 128, D], f16, kind="ExternalInput")
    c128_d = nc.dram_tensor("c128", [128, CW], f16, kind="ExternalInput")
    c32_d = nc.dram_tensor("c32", [TPG, 129], f16, kind="ExternalInput")
    h_d = nc.dram_tensor("h", [TOK, D], f16, kind="ExternalOutput")

    vA = slabA_d.ap()
    vB = slabB_d.ap()
    vP = slabP_d.ap()
    hout = h_d.ap()

    with tile.TileContext(nc) as tc:
        import contextlib
        ctx = contextlib.ExitStack()
        with ctx:
            consts = ctx.enter_context(tc.tile_pool(name="consts", bufs=1))
            pA = ctx.enter_context(tc.tile_pool(name="pA", bufs=NSQ))
            pB = ctx.enter_context(tc.tile_pool(name="pB", bufs=NSQ))
            pP = ctx.enter_context(tc.tile_pool(name="pP", bufs=NSQ))
            stats_pool = ctx.enter_context(tc.tile_pool(name="stats", bufs=3))
            sm_pool = ctx.enter_context(tc.tile_pool(name="sm", bufs=2))
            hsb_pool = ctx.enter_context(tc.tile_pool(name="hsb", bufs=3))
            hpage_pool = ctx.enter_context(
                tc.tile_pool(name="hpage", bufs=3, space="PSUM"))
            z_pool = ctx.enter_context(
                tc.tile_pool(name="zp", bufs=1, space="PSUM"))
            rzb_pool = ctx.enter_context(
                tc.tile_pool(name="rzb", bufs=1, space="PSUM"))

            # ---- const tiles + scratch (DMAs issued after SQ0 slabs) ----
            c128 = consts.tile([128, CW], f16)
            w2b = c128[:, 0:D]
            ohA = c128[:, D:D + TPG]
            ohA8 = c128[:, D + TPG:D + TPG + 8 * TPG]
            gsel = c128[:, D + 9 * TPG:D + 9 * TPG + NG]
            ieye = c128[:, D + 9 * TPG + NG:CW]
            c32 = consts.tile([TPG, 129], f16)
            ohAT = c32[:, 0:128]
            ones32 = c32[:, 128:129]
            eps_col = consts.tile([128, 1], f32)
            nc.vector.memset(eps_col[:], EPS)
            zero_col = consts.tile([128, 1], f32)
            nc.vector.memset(zero_col[:], 0.0)
            # elementwise-output scratch (values never read; overwritten
            # in program order on each engine)
            scrA = consts.tile([128, D], f16)
            scrD = consts.tile([128, D], f16)

            # ---- input DMA: everything prefetched up-front.
            # SQ0 (chunked) first so compute starts ASAP, then consts,
            # then the rest; SQ4-7 go via the (otherwise idle) gpsimd
            # SWDGE queue so the sync queue's issue serialization does
            # not delay them.
            slabA_t, slabB_t, slabP_t = [], [], []
            for sq in range(NSQ):
                slabA_t.append(pA.tile([128, NG * D], f16, tag="slabA",
                                       name=f"slabA{sq}"))
                slabB_t.append(pB.tile([128, NG * D], f16, tag="slabB",
                                       name=f"slabB{sq}"))
                slabP_t.append(pP.tile([128, D], f16, tag="slabP",
                                       name=f"slabP{sq}"))

            def issue_slab(sq, nch, eng):
                ta, tb, tp = slabA_t[sq], slabB_t[sq], slabP_t[sq]
                cw = NG * D // nch
                for ci in range(nch):
                    sl = slice(ci * cw, (ci + 1) * cw)
                    eng.dma_start(ta[:, sl], vA[sq][:, sl])
                for ci in range(nch):
                    sl = slice(ci * cw, (ci + 1) * cw)
                    eng.dma_start(tb[:, sl], vB[sq][:, sl])
                eng.dma_start(tp[:, :], vP[sq][:, :])

            issue_slab(0, 4, nc.sync)
            nc.sync.dma_start(c128[:], c128_d.ap()[:])
            nc.sync.dma_start(c32[:], c32_d.ap()[:])
            issue_slab(1, 2, nc.sync)
            for sq in range(2, 4):
                issue_slab(sq, 1, nc.sync)
            for sq in range(4, NSQ):
                issue_slab(sq, 1, nc.gpsimd)

            state = {}

            def emit_stats(i):
                st = stats_pool.tile([128, 18], f32, tag="stats")
                state[i] = {"stats": st}
                # ssq units (9): ACT Square+accum (last SSQ_DVE of them on DVE)
                units = [(slabA_t[i], g * D, g) for g in range(NG)] \
                    + [(slabB_t[i], g * D, 4 + g) for g in range(NG)] \
                    + [(slabP_t[i], 0, 8)]
                for t, c0, sc in units[:9 - SSQ_DVE]:
                    nc.scalar.activation(
                        scrA[:, :], t[:, c0:c0 + D], AF.Square,
                        bias=zero_col[:], accum_out=st[:, sc:sc + 1])
                for t, c0, sc in units[9 - SSQ_DVE:]:
                    nc.vector.scalar_tensor_tensor(
                        out=scrD[:, :], in0=t[:, c0:c0 + D],
                        scalar=1.0, in1=t[:, c0:c0 + D],
                        op0=OP.mult, op1=OP.mult,
                        accum_out=st[:, sc:sc + 1])
                # dot units (9): DVE stt with w2b
                for t, c0, sc in units:
                    nc.vector.scalar_tensor_tensor(
                        out=scrD[:, :], in0=t[:, c0:c0 + D],
                        scalar=1.0, in1=w2b,
                        op0=OP.mult, op1=OP.mult,
                        accum_out=st[:, 9 + sc:10 + sc])

            def emit_softmax(i):
                st = state[i]["stats"]
                lnq = sm_pool.tile([128, 9], f32, tag="lnq")
                nc.scalar.activation(lnq[:], st[:, 0:9], AF.Ln,
                                     bias=eps_col[:], scale=1.0 / D)
                rq = sm_pool.tile([128, 9], f32, tag="rq")
                nc.scalar.activation(rq[:], lnq[:], AF.Exp,
                                     bias=zero_col[:], scale=-0.5)
                lg = sm_pool.tile([128, 9], f32, tag="lg")
                nc.vector.tensor_tensor(out=lg[:], in0=st[:, 9:18],
                                        in1=rq[:], op=OP.mult)
                e_all = sm_pool.tile([128, 9], f16, tag="e_all")
                nc.scalar.activation(e_all[:], lg[:], AF.Exp,
                                     bias=zero_col[:])
                e8m = sm_pool.tile([128, TPG], f16, tag="e8m")
                nc.vector.tensor_tensor(
                    out=e8m[:], in0=ohA,
                    in1=e_all[:, 8:9].to_broadcast([128, TPG]), op=OP.mult)

                zp = z_pool.tile([TPG, NG], f32)
                nc.tensor.matmul(zp[:], lhsT=ohA, rhs=e_all[:, 0:4],
                                 start=True, stop=False)
                nc.tensor.matmul(zp[:], lhsT=ohA, rhs=e_all[:, 4:8],
                                 start=False, stop=False)
                nc.tensor.matmul(zp[:], lhsT=e8m[:], rhs=gsel,
                                 start=False, stop=True)
                rz = sm_pool.tile([TPG, NG], f32, tag="rz")
                nc.vector.reciprocal(rz[:], zp[:])
                # rzcol[p = g*32+t'] = rz[t', g] via rzsel matmul
                rzsel = sm_pool.tile([TPG, 128], f16, tag="rzsel")
                nc.vector.tensor_tensor(
                    out=rzsel[:, :].rearrange("q (g j) -> q g j", g=NG),
                    in0=ohAT.rearrange("q (g j) -> q g j", g=NG),
                    in1=rz[:, :].unsqueeze(2).to_broadcast([TPG, NG, TPG]),
                    op=OP.mult)
                rzb = rzb_pool.tile([128, 1], f32)
                nc.tensor.matmul(rzb[:], lhsT=rzsel[:], rhs=ones32,
                                 start=True, stop=True)
                rzcol = sm_pool.tile([128, 1], f32, tag="rzcol")
                nc.vector.tensor_copy(rzcol[:], rzb[:])

                # unnormalized weight masks from raw e values
                lhsTAB = sm_pool.tile([128, 8 * TPG], f16, tag="lhsTAB")
                nc.vector.tensor_tensor(
                    out=lhsTAB[:, :].rearrange("p (g j) -> p g j", g=8),
                    in0=ohA8.rearrange("p (g j) -> p g j", g=8),
                    in1=e_all[:, 0:8].unsqueeze(2).to_broadcast([128, 8, TPG]),
                    op=OP.mult)
                lhsTPd = sm_pool.tile([128, 128], f16, tag="lhsTPd")
                nc.vector.tensor_tensor(
                    out=lhsTPd[:], in0=ieye,
                    in1=e_all[:, 8:9].to_broadcast([128, 128]), op=OP.mult)

                hpage = hpage_pool.tile([128, D], f32, tag="hpage",
                                        name="hpage")
                for g in range(NG):
                    col = g * TPG
                    for hh in range(2):
                        osl = slice(512 * hh, 512 * hh + 512)
                        nc.tensor.matmul(
                            hpage[col:col + TPG, osl],
                            lhsT=lhsTAB[:, col:col + TPG],
                            rhs=slabA_t[i][:, g * D + 512 * hh:
                                           g * D + 512 * hh + 512],
                            start=True, stop=False, tile_position=(0, col))
                        nc.tensor.matmul(
                            hpage[col:col + TPG, osl],
                            lhsT=lhsTAB[:, 128 + col:128 + col + TPG],
                            rhs=slabB_t[i][:, g * D + 512 * hh:
                                           g * D + 512 * hh + 512],
                            start=False, stop=False, tile_position=(0, col))
                        nc.tensor.matmul(
                            hpage[col:col + TPG, osl],
                            lhsT=lhsTPd[:, col:col + TPG],
                            rhs=slabP_t[i][:, osl],
                            start=False, stop=True, tile_position=(0, col))
                state[i]["hpage"] = hpage
                state[i]["rzcol"] = rzcol

            def emit_out(i):
                hpage = state[i]["hpage"]
                rzcol = state[i]["rzcol"]
                h_sb = hsb_pool.tile([128, D], f16, tag="h_sb")
                if COPY_DVE > 0:
                    nc.vector.tensor_scalar_mul(
                        h_sb[:, 0:COPY_DVE], hpage[:, 0:COPY_DVE], rzcol[:])
                if COPY_DVE < D:
                    nc.scalar.activation(h_sb[:, COPY_DVE:D],
                                         hpage[:, COPY_DVE:D],
                                         AF.Copy, scale=rzcol[:])
                nc.gpsimd.dma_start(hout[i * SQTOK:(i + 1) * SQTOK, :],
                                    h_sb[:, :])
                del state[i]

            for i in range(NSQ + 2):
                if i < NSQ:
                    emit_stats(i)
                if 1 <= i <= NSQ:
                    emit_softmax(i - 1)
                if i >= 2:
                    emit_out(i - 2)

    nc.compile()
    return nc


def _host_inputs(blocks, partial_block, proj_w, norm_w):
    """Slice + rearrange per-core inputs (host-side, numpy only)."""
    if DTYPE == "bf16":
        import ml_dtypes
        npdt = np.dtype(ml_dtypes.bfloat16)
    else:
        npdt = np.dtype(np.float16)
    blocks16 = np.asarray(blocks, np.float32).astype(npdt).reshape(N, B * T, D)
    partial16 = np.asarray(partial_block, np.float32).astype(npdt).reshape(
        B * T, D)
    w2 = (np.asarray(proj_w, np.float32)
          * np.asarray(norm_w, np.float32)).astype(npdt)
    w2b = np.broadcast_to(w2, (128, D))
    p = np.arange(128)
    ohA = (p[:, None] % TPG == np.arange(TPG)[None, :]).astype(npdt)
    ohA8 = np.tile(ohA, (1, 8))
    gsel = (p[:, None] // TPG == np.arange(NG)[None, :]).astype(npdt)
    ieye = np.eye(128).astype(npdt)
    c128 = np.ascontiguousarray(
        np.concatenate([w2b, ohA, ohA8, gsel, ieye], axis=1))
    c32 = np.ascontiguousarray(
        np.concatenate([ohA.T, np.ones((TPG, 1), npdt)], axis=1))

    in_maps = []
    for c in range(N_CORES):
        s = slice(c * TOK, (c + 1) * TOK)
        # slabA[sq, n*32+t', g*1024+d] = blocks[n, sq*128+g*32+t', d]
        ba = blocks16[0:4, s].reshape(4, NSQ, NG, TPG, D)
        slabA = np.ascontiguousarray(
            ba.transpose(1, 0, 3, 2, 4)).reshape(NSQ, 128, NG * D)
        bb = blocks16[4:8, s].reshape(4, NSQ, NG, TPG, D)
        slabB = np.ascontiguousarray(
            bb.transpose(1, 0, 3, 2, 4)).reshape(NSQ, 128, NG * D)
        slabP = np.ascontiguousarray(partial16[s].reshape(NSQ, 128, D))
        in_maps.append({
            "slabA": slabA,
            "slabB": slabB,
            "slabP": slabP,
            "c128": c128,
            "c32": c32,
        })
    return in_maps


def kernel(blocks, partial_block, proj_w, norm_w):
    from concourse.bass_utils import run_bass_kernel_spmd

    if "nc" not in _CACHE:
        _CACHE["nc"] = build_nc()
    nc = _CACHE["nc"]
    in_maps = _host_inputs(blocks, partial_block, proj_w, norm_w)
    res = run_bass_kernel_spmd(nc, in_maps, core_ids=list(range(N_CORES)))
    h = np.concatenate([np.asarray(res.results[c]["h"])
                        for c in range(N_CORES)], axis=0)
    return h.astype(np.float32).reshape(B, T, D)


# revision 17
# speedup vs baseline: 1.1866x; 1.1866x over previous
"""BlockAttentionResidual Trainium2 kernel (v2: fp16, 32-token groups).

Math per token t over NB=9 blocks (8 full + 1 partial), D=1024:
    rq[n,t]   = (ssq[n,t]/D + eps)^(-1/2)
    logit     = (sum_d w2[d]*v[n,t,d]) * rq,   w2 = proj_w*norm_w
    w[n,t]    = softmax_n(logit)
    h[t,d]    = sum_n w[n,t]*v[n,t,d]

Sharding: B*T = 8192 tokens -> 1024 tokens/core on 8 cores.
Per core: 8 superquads (SQ) of 128 tokens = 4 groups x 32 tokens.
fp16 data path (host-side cast; harness tolerance is 2e-2, fp16 keeps
rel err ~1e-3), fp32 accumulation in PSUM / stats.

Layout per SQ (partition dim first):
  slabA[p = n*32+t', c = g*1024+d]  n in 0..3   [128, 4096]
  slabB: same for n in 4..7                      [128, 4096]
  slabP[p = g*32+t', d]  partial block           [128, 1024]

Stats: ssq via ACT Square+accum_out, dot via DVE stt+accum_out, into
stats[:, 0:9]=ssq / [:, 9:18]=dot (order: A g0-3, B g0-3, P).
Softmax over n via PE one-hot matmuls (Z accumulated in PSUM):
  Z[t',g] = ohA^T@eA + ohA^T@eB + (ohA*e8)^T@gsel ; rz = 1/Z
h accumulated UNNORMALIZED in PSUM (weights = raw e values):
  per (group, 512-col half): 3 accumulated matmuls
  (lhsT = ohA8*e masks for A/B; diag(e8) built from I128 for P).
Normalization by 1/Z happens in the PSUM->SBUF copy:
  rzcol[p = g*32+t'] = rz[t',g] (via rzsel matmul), then
  h_sb = hpage * rzcol (ACT Copy with scale AP / DVE tensor_scalar).
PSUM pages are fully packed (128 tokens/page) so the copy and the
output DMA are contiguous [128, 1024].
Three-stage software pipeline: stats(i) | softmax+mm(i-1) | copy+out(i-2).
"""

import os
import sys
import numpy as np

for _p in ("/opt/trn_rl_repo", "/root/.axon_site/_ro/trn_rl_repo"):
    if os.path.isdir(_p) and _p not in sys.path:
        sys.path.append(_p)

N_CORES = 8
N, B, T, D = 8, 4, 2048, 1024
EPS = 1e-6
TOK = (B * T) // N_CORES          # 1024 tokens per core
TPG = 32                          # tokens per group
NG = 4                            # groups per superquad
SQTOK = TPG * NG                  # 128 tokens per superquad
NSQ = TOK // SQTOK                # 8 superquads per core

# knobs for ACT/DVE balance
COPY_DVE = int(os.environ.get("BLOCKATTN_COPY_DVE", "448"))
SSQ_DVE = int(os.environ.get("BLOCKATTN_SSQ_DVE", "0"))  # of 9 ssq units -> DVE
DTYPE = os.environ.get("BLOCKATTN_DTYPE", "bf16")  # bf16 | fp16
ACT_SET = "natural_log_exp_and_others"

_CACHE = {}


def _patch_act_tables():
    """Make every activation func this kernel uses resolve to one table set
    (ACT_SET), so bacc emits a single ACT_TABLE_LOAD instead of thrashing
    between sets on every Ln/Exp/Square transition."""
    import concourse.bacc as bacc_mod
    import concourse.hw_specs as hw_specs
    from concourse import mybir

    if getattr(bacc_mod, "_blockattn_act_patch", False):
        return
    AF = mybir.ActivationFunctionType
    mine = {AF.Square, AF.Exp, AF.Ln, AF.Copy, AF.Identity}
    orig = hw_specs.get_activation_tables

    def patched(arch):
        t = dict(orig(arch))
        assert ACT_SET in t and mine <= t[ACT_SET], (ACT_SET, t.get(ACT_SET))
        return {
            name: (funcs if name == ACT_SET else funcs - mine)
            for name, funcs in t.items()
        }

    bacc_mod.get_activation_tables = patched
    bacc_mod._blockattn_act_patch = True


def build_nc():
    import concourse.bacc as bacc
    import concourse.tile as tile
    from concourse import mybir

    _patch_act_tables()

    f32 = mybir.dt.float32
    f16 = mybir.dt.bfloat16 if DTYPE == "bf16" else mybir.dt.float16
    AF = mybir.ActivationFunctionType
    OP = mybir.AluOpType

    nc = bacc.Bacc("TRN2", target_bir_lowering=False, debug=False)

    CW = D + TPG + 8 * TPG + NG + 128  # packed 128-row consts
    slabA_d = nc.dram_tensor("slabA", [NSQ, 128, NG * D], f16, kind="ExternalInput")
    slabB_d = nc.dram_tensor("slabB", [NSQ, 128, NG * D], f16, kind="ExternalInput")
    slabP_d = nc.dram_tensor("slabP", [NSQ, 128, D], f16, kind="ExternalInput")
    c128_d = nc.dram_tensor("c128", [128, CW], f16, kind="ExternalInput")
    c32_d = nc.dram_tensor("c32", [TPG, 129], f16, kind="ExternalInput")
    h_d = nc.dram_tensor("h", [TOK, D], f16, kind="ExternalOutput")

    vA = slabA_d.ap()
    vB = slabB_d.ap()
    vP = slabP_d.ap()
    hout = h_d.ap()

    with tile.TileContext(nc) as tc:
        import contextlib
        ctx = contextlib.ExitStack()
        with ctx:
            consts = ctx.enter_context(tc.tile_pool(name="consts", bufs=1))
            pA = ctx.enter_context(tc.tile_pool(name="pA", bufs=NSQ))
            pB = ctx.enter_context(tc.tile_pool(name="pB", bufs=NSQ))
            pP = ctx.enter_context(tc.tile_pool(name="pP", bufs=NSQ))
            stats_pool = ctx.enter_context(tc.tile_pool(name="stats", bufs=3))
            sm_pool = ctx.enter_context(tc.tile_pool(name="sm", bufs=2))
            hsb_pool = ctx.enter_context(tc.tile_pool(name="hsb", bufs=3))
            hpage_pool = ctx.enter_context(
                tc.tile_pool(name="hpage", bufs=3, space="PSUM"))
            z_pool = ctx.enter_context(
                tc.tile_pool(name="zp", bufs=1, space="PSUM"))
            rzb_pool = ctx.enter_context(
                tc.tile_pool(name="rzb", bufs=1, space="PSUM"))

            # ---- const tiles + scratch (DMAs issued after SQ0 slabs) ----
            c128 = consts.tile([128, CW], f16)
            w2b = c128[:, 0:D]
            ohA = c128[:, D:D + TPG]
            ohA8 = c128[:, D + TPG:D + TPG + 8 * TPG]
            gsel = c128[:, D + 9 * TPG:D + 9 * TPG + NG]
            ieye = c128[:, D + 9 * TPG + NG:CW]
            c32 = consts.tile([TPG, 129], f16)
            ohAT = c32[:, 0:128]
            ones32 = c32[:, 128:129]
            eps_col = consts.tile([128, 1], f32)
            nc.vector.memset(eps_col[:], EPS)
            zero_col = consts.tile([128, 1], f32)
            nc.vector.memset(zero_col[:], 0.0)
            # elementwise-output scratch (values never read; overwritten
            # in program order on each engine)
            scrA = consts.tile([128, D], f16)
            scrD = consts.tile([128, D], f16)

            # ---- input DMA: everything prefetched up-front.
            # SQ0 (chunked) first so compute starts ASAP, then consts,
            # then the rest; SQ4-7 go via the (otherwise idle) gpsimd
            # SWDGE queue so the sync queue's issue serialization does
            # not delay them.
            slabA_t, slabB_t, slabP_t = [], [], []
            for sq in range(NSQ):
                slabA_t.append(pA.tile([128, NG * D], f16, tag="slabA",
                                       name=f"slabA{sq}"))
                slabB_t.append(pB.tile([128, NG * D], f16, tag="slabB",
                                       name=f"slabB{sq}"))
                slabP_t.append(pP.tile([128, D], f16, tag="slabP",
                                       name=f"slabP{sq}"))

            def issue_slab(sq, nch, eng):
                ta, tb, tp = slabA_t[sq], slabB_t[sq], slabP_t[sq]
                cw = NG * D // nch
                for ci in range(nch):
                    sl = slice(ci * cw, (ci + 1) * cw)
                    eng.dma_start(ta[:, sl], vA[sq][:, sl])
                for ci in range(nch):
                    sl = slice(ci * cw, (ci + 1) * cw)
                    eng.dma_start(tb[:, sl], vB[sq][:, sl])
                eng.dma_start(tp[:, :], vP[sq][:, :])

            issue_slab(0, 4, nc.sync)
            nc.sync.dma_start(c128[:], c128_d.ap()[:])
            nc.sync.dma_start(c32[:], c32_d.ap()[:])
            issue_slab(1, 2, nc.sync)
            for sq in range(2, NSQ):
                issue_slab(sq, 1, nc.sync)

            state = {}

            def emit_stats(i):
                st = stats_pool.tile([128, 18], f32, tag="stats")
                state[i] = {"stats": st}
                # ssq units (9): ACT Square+accum (last SSQ_DVE of them on DVE)
                units = [(slabA_t[i], g * D, g) for g in range(NG)] \
                    + [(slabB_t[i], g * D, 4 + g) for g in range(NG)] \
                    + [(slabP_t[i], 0, 8)]
                for t, c0, sc in units[:9 - SSQ_DVE]:
                    nc.scalar.activation(
                        scrA[:, :], t[:, c0:c0 + D], AF.Square,
                        bias=zero_col[:], accum_out=st[:, sc:sc + 1])
                for t, c0, sc in units[9 - SSQ_DVE:]:
                    nc.vector.scalar_tensor_tensor(
                        out=scrD[:, :], in0=t[:, c0:c0 + D],
                        scalar=1.0, in1=t[:, c0:c0 + D],
                        op0=OP.mult, op1=OP.mult,
                        accum_out=st[:, sc:sc + 1])
                # dot units (9): DVE stt with w2b
                for t, c0, sc in units:
                    nc.vector.scalar_tensor_tensor(
                        out=scrD[:, :], in0=t[:, c0:c0 + D],
                        scalar=1.0, in1=w2b,
                        op0=OP.mult, op1=OP.mult,
                        accum_out=st[:, 9 + sc:10 + sc])

            def emit_softmax(i):
                st = state[i]["stats"]
                lnq = sm_pool.tile([128, 9], f32, tag="lnq")
                nc.scalar.activation(lnq[:], st[:, 0:9], AF.Ln,
                                     bias=eps_col[:], scale=1.0 / D)
                rq = sm_pool.tile([128, 9], f32, tag="rq")
                nc.scalar.activation(rq[:], lnq[:], AF.Exp,
                                     bias=zero_col[:], scale=-0.5)
                lg = sm_pool.tile([128, 9], f32, tag="lg")
                nc.vector.tensor_tensor(out=lg[:], in0=st[:, 9:18],
                                        in1=rq[:], op=OP.mult)
                e_all = sm_pool.tile([128, 9], f16, tag="e_all")
                nc.scalar.activation(e_all[:], lg[:], AF.Exp,
                                     bias=zero_col[:])
                e8m = sm_pool.tile([128, TPG], f16, tag="e8m")
                nc.vector.tensor_tensor(
                    out=e8m[:], in0=ohA,
                    in1=e_all[:, 8:9].to_broadcast([128, TPG]), op=OP.mult)

                zp = z_pool.tile([TPG, NG], f32)
                nc.tensor.matmul(zp[:], lhsT=ohA, rhs=e_all[:, 0:4],
                                 start=True, stop=False)
                nc.tensor.matmul(zp[:], lhsT=ohA, rhs=e_all[:, 4:8],
                                 start=False, stop=False)
                nc.tensor.matmul(zp[:], lhsT=e8m[:], rhs=gsel,
                                 start=False, stop=True)
                rz = sm_pool.tile([TPG, NG], f32, tag="rz")
                nc.vector.reciprocal(rz[:], zp[:])
                # rzcol[p = g*32+t'] = rz[t', g] via rzsel matmul
                rzsel = sm_pool.tile([TPG, 128], f16, tag="rzsel")
                nc.vector.tensor_tensor(
                    out=rzsel[:, :].rearrange("q (g j) -> q g j", g=NG),
                    in0=ohAT.rearrange("q (g j) -> q g j", g=NG),
                    in1=rz[:, :].unsqueeze(2).to_broadcast([TPG, NG, TPG]),
                    op=OP.mult)
                rzb = rzb_pool.tile([128, 1], f32)
                nc.tensor.matmul(rzb[:], lhsT=rzsel[:], rhs=ones32,
                                 start=True, stop=True)
                rzcol = sm_pool.tile([128, 1], f32, tag="rzcol")
                nc.vector.tensor_copy(rzcol[:], rzb[:])

                # unnormalized weight masks from raw e values
                lhsTAB = sm_pool.tile([128, 8 * TPG], f16, tag="lhsTAB")
                nc.vector.tensor_tensor(
                    out=lhsTAB[:, :].rearrange("p (g j) -> p g j", g=8),
                    in0=ohA8.rearrange("p (g j) -> p g j", g=8),
                    in1=e_all[:, 0:8].unsqueeze(2).to_broadcast([128, 8, TPG]),
                    op=OP.mult)
                lhsTPd = sm_pool.tile([128, 128], f16, tag="lhsTPd")
                nc.vector.tensor_tensor(
                    out=lhsTPd[:], in0=ieye,
                    in1=e_all[:, 8:9].to_broadcast([128, 128]), op=OP.mult)

                hpage = hpage_pool.tile([128, D], f32, tag="hpage",
                                        name="hpage")
                for g in range(NG):
                    col = g * TPG
                    for hh in range(2):
                        osl = slice(512 * hh, 512 * hh + 512)
                        nc.tensor.matmul(
                            hpage[col:col + TPG, osl],
                            lhsT=lhsTAB[:, col:col + TPG],
                            rhs=slabA_t[i][:, g * D + 512 * hh:
                                           g * D + 512 * hh + 512],
                            start=True, stop=False, tile_position=(0, col))
                        nc.tensor.matmul(
                            hpage[col:col + TPG, osl],
                            lhsT=lhsTAB[:, 128 + col:128 + col + TPG],
                            rhs=slabB_t[i][:, g * D + 512 * hh:
                                           g * D + 512 * hh + 512],
                            start=False, stop=False, tile_position=(0, col))
                        nc.tensor.matmul(
                            hpage[col:col + TPG, osl],
                            lhsT=lhsTPd[:, col:col + TPG],
                            rhs=slabP_t[i][:, osl],
                            start=False, stop=True, tile_position=(0, col))
                state[i]["hpage"] = hpage
                state[i]["rzcol"] = rzcol

            def emit_out(i):
                hpage = state[i]["hpage"]
                rzcol = state[i]["rzcol"]
                h_sb = hsb_pool.tile([128, D], f16, tag="h_sb")
                if COPY_DVE > 0:
                    nc.vector.tensor_scalar_mul(
                        h_sb[:, 0:COPY_DVE], hpage[:, 0:COPY_DVE], rzcol[:])
                if COPY_DVE < D:
                    nc.scalar.activation(h_sb[:, COPY_DVE:D],
                                         hpage[:, COPY_DVE:D],
                                         AF.Copy, scale=rzcol[:])
                nc.gpsimd.dma_start(hout[i * SQTOK:(i + 1) * SQTOK, :],
                                    h_sb[:, :])
                del state[i]

            for i in range(NSQ + 2):
                if i < NSQ:
                    emit_stats(i)
                if 1 <= i <= NSQ:
                    emit_softmax(i - 1)
                if i >= 2:
                    emit_out(i - 2)

    nc.compile()
    return nc


def _host_inputs(blocks, partial_block, proj_w, norm_w):
    """Slice + rearrange per-core inputs (host-side, numpy only)."""
    if DTYPE == "bf16":
        import ml_dtypes
        npdt = np.dtype(ml_dtypes.bfloat16)
    else:
        npdt = np.dtype(np.float16)
    blocks16 = np.asarray(blocks, np.float32).astype(npdt).reshape(N, B * T, D)
    partial16 = np.asarray(partial_block, np.float32).astype(npdt).reshape(
        B * T, D)
    w2 = (np.asarray(proj_w, np.float32)
          * np.asarray(norm_w, np.float32)).astype(npdt)
    w2b = np.broadcast_to(w2, (128, D))
    p = np.arange(128)
    ohA = (p[:, None] % TPG == np.arange(TPG)[None, :]).astype(npdt)
    ohA8 = np.tile(ohA, (1, 8))
    gsel = (p[:, None] // TPG == np.arange(NG)[None, :]).astype(npdt)
    ieye = np.eye(128).astype(npdt)
    c128 = np.ascontiguousarray(
        np.concatenate([w2b, ohA, ohA8, gsel, ieye], axis=1))
    c32 = np.ascontiguousarray(
        np.concatenate([ohA.T, np.ones((TPG, 1), npdt)], axis=1))

    in_maps = []
    for c in range(N_CORES):
        s = slice(c * TOK, (c + 1) * TOK)
        # slabA[sq, n*32+t', g*1024+d] = blocks[n, sq*128+g*32+t', d]
        ba = blocks16[0:4, s].reshape(4, NSQ, NG, TPG, D)
        slabA = np.ascontiguousarray(
            ba.transpose(1, 0, 3, 2, 4)).reshape(NSQ, 128, NG * D)
        bb = blocks16[4:8, s].reshape(4, NSQ, NG, TPG, D)
        slabB = np.ascontiguousarray(
            bb.transpose(1, 0, 3, 2, 4)).reshape(NSQ, 128, NG * D)
        slabP = np.ascontiguousarray(partial16[s].reshape(NSQ, 128, D))
        in_maps.append({
            "slabA": slabA,
            "slabB": slabB,
            "slabP": slabP,
            "c128": c128,
            "c32": c32,
        })
    return in_maps


def kernel(blocks, partial_block, proj_w, norm_w):
    from concourse.bass_utils import run_bass_kernel_spmd

    if "nc" not in _CACHE:
        _CACHE["nc"] = build_nc()
    nc = _CACHE["nc"]
    in_maps = _host_inputs(blocks, partial_block, proj_w, norm_w)
    res = run_bass_kernel_spmd(nc, in_maps, core_ids=list(range(N_CORES)))
    h = np.concatenate([np.asarray(res.results[c]["h"])
                        for c in range(N_CORES)], axis=0)
    return h.astype(np.float32).reshape(B, T, D)
